# revision 17
# baseline (speedup 1.0000x reference)
"""ClusterInversionLoss Trainium2 kernel.

Strategy (data-parallel over the flat pair list, per sharding hint):
  - Host: co-locate each pair's data by gathering rows at pair_i/pair_j
    (l0-shifted logits per class, target delta, sample weights), shard
    the 2M pairs evenly across 8 cores, per-core planes (128, 11, L) bf16.
  - Device (per core, pure streaming, no random access):
      s = softmax-expected-score for both pair sides: exp on ACT (one
      table set, pinned), Z and W = sum c*e_c via a shared suffix-sum
      add chain in bf16 on DVE, 1/Z = exp(-ln Z) on ACT, pair chain on
      the Pool engine, softplus via exp/ln with the free affine bias,
      fused multiply + per-partition reduce via scalar_tensor_tensor.
  - Host: sum the 8x128 partial (loss, weight) pairs, return the ratio.

Computes exactly the reference quantity; only rows referenced by pairs
contribute to the loss, so unpaired rows need not be touched.
"""

import numpy as np

import concourse.bacc as bacc
import concourse.mybir as mybir
from concourse.bass_utils import run_bass_kernel_spmd
from concourse.tile import TileContext

NCORES = 8
NPAIRS = 2_000_000
PC = NPAIRS // NCORES  # 250_000 pairs per core
P = 128
NCHUNK = 2
LC = 978
L = NCHUNK * LC  # 1956 columns per partition; P*L = 250_368 >= PC
NPLANE = 11  # li1'..li4', lj1'..lj4' (l0-shifted logits), dy, wi, wj

EPS = 1e-8

f32 = mybir.dt.float32
bf16 = mybir.dt.bfloat16
AF = mybir.ActivationFunctionType
ALU = mybir.AluOpType


def _pin_act_tables(arch):
    """Make every ACT function we use first-match to one table set that
    contains both exp and ln, so the kernel needs a single
    ACT_TABLE_LOAD instead of thrashing between the exp-only and
    ln-only sets (1.3us per reload).  Only membership of the cached
    selection dict is edited; set indices (act_func_set_id) and the
    real on-device tables are untouched, so lowering stays correct.
    """
    from concourse.hw_specs import get_activation_tables

    tabs = get_activation_tables(arch)
    ours = {AF.Exp, AF.Ln, AF.Sign, AF.Abs, AF.Square}
    combined = None
    for name, fns in tabs.items():
        if ours <= fns:
            combined = name
            break
    if combined is None:
        return
    for name, fns in tabs.items():
        if name != combined:
            fns -= ours


def _build():
    nc = bacc.Bacc("TRN2", target_bir_lowering=False)
    _pin_act_tables(nc.m.arch)
    X = nc.dram_tensor("x", [P, NPLANE, L], bf16, kind="ExternalInput")
    OUT = nc.dram_tensor("out", [P, 2], f32, kind="ExternalOutput")

    with TileContext(nc) as tc:
        with (
            tc.tile_pool(name="io", bufs=2) as io,
            tc.tile_pool(name="ew", bufs=2) as ew,
            tc.tile_pool(name="sc", bufs=2) as sc,
            tc.tile_pool(name="s1", bufs=1) as s1p,
            tc.tile_pool(name="acc", bufs=1) as accp,
        ):
            accL = [accp.tile([P, 1], f32, tag=f"accL{c}", name=f"accL{c}")
                    for c in range(NCHUNK)]
            accW = [accp.tile([P, 1], f32, tag=f"accW{c}", name=f"accW{c}")
                    for c in range(NCHUNK)]
            ST = [{} for _ in range(NCHUNK)]

            def stage1(c):
                """DMA, exp, suffix-sum Z/W, reciprocal-of-Z."""
                t = ST[c]
                cs = slice(c * LC, (c + 1) * LC)
                LI = io.tile([P, 4, LC], bf16, tag="LI", name="LI")
                nc.sync.dma_start(out=LI[:], in_=X[:, 0:4, cs])
                LJ = io.tile([P, 4, LC], bf16, tag="LJ", name="LJ")
                nc.sync.dma_start(out=LJ[:], in_=X[:, 4:8, cs])
                DY = io.tile([P, LC], bf16, tag="DY", name="DY")
                nc.sync.dma_start(out=DY[:], in_=X[:, 8, cs])
                WI = io.tile([P, LC], bf16, tag="WI", name="WI")
                nc.sync.dma_start(out=WI[:], in_=X[:, 9, cs])
                WJ = io.tile([P, LC], bf16, tag="WJ", name="WJ")
                nc.sync.dma_start(out=WJ[:], in_=X[:, 10, cs])
                t.update(DY=DY, WI=WI, WJ=WJ)

                EI = ew.tile([P, 4, LC], bf16, tag="EI", name="EI")
                nc.scalar.activation(EI[:], LI[:], AF.Exp)
                EJ = ew.tile([P, 4, LC], bf16, tag="EJ", name="EJ")
                nc.scalar.activation(EJ[:], LJ[:], AF.Exp)

                # suffix-sum chains: A=e3+e4; B=e2+A; T1=e1+B; Z=1+T1;
                # U=T1+B; V=A+e4; W=U+V = e1+2e2+3e3+4e4
                ZIJ = sc.tile([P, 2, LC], bf16, tag="ZIJ", name="ZIJ")
                WT = {}
                for side, (E, an, bn, tn) in enumerate(
                        ((EI, "Ai", "Bi", "Ti"), (EJ, "Aj", "Bj", "Tj"))):
                    A = sc.tile([P, LC], bf16, tag=an, name=an)
                    B = sc.tile([P, LC], bf16, tag=bn, name=bn)
                    T = sc.tile([P, LC], bf16, tag=tn, name=tn)
                    nc.vector.tensor_add(out=A[:], in0=E[:, 2, :], in1=E[:, 3, :])
                    nc.vector.tensor_add(out=B[:], in0=E[:, 1, :], in1=A[:])
                    nc.vector.tensor_add(out=T[:], in0=E[:, 0, :], in1=B[:])
                    nc.vector.tensor_scalar_add(out=ZIJ[:, side, :], in0=T[:],
                                                scalar1=1.0)
                    nc.vector.tensor_add(out=B[:], in0=T[:], in1=B[:])
                    nc.vector.tensor_add(out=A[:], in0=A[:], in1=E[:, 3, :])
                    nc.vector.tensor_add(out=T[:], in0=B[:], in1=A[:])
                    WT[side] = T
                t["WT"] = WT

                ZT = s1p.tile([P, 2, LC], f32, tag="ZT", name="ZT")
                nc.scalar.activation(ZT[:], ZIJ[:], AF.Ln)
                RZ = s1p.tile([P, 2, LC], f32, tag="RZ", name="RZ")
                nc.scalar.activation(RZ[:], ZT[:], AF.Exp, scale=-1.0)
                t["RZ"] = RZ

            def stage2(c):
                """sign/abs, pair chain on Pool, softplus, fused reduces."""
                t = ST[c]
                DY, WI, WJ, RZ, WT = t["DY"], t["WI"], t["WJ"], t["RZ"], t["WT"]
                SG = s1p.tile([P, LC], f32, tag="SG", name="SG")
                nc.scalar.activation(SG[:], DY[:], AF.Sign)
                DIST = s1p.tile([P, LC], f32, tag="DIST", name="DIST")
                nc.scalar.activation(DIST[:], DY[:], AF.Abs)

                Si = s1p.tile([P, LC], f32, tag="Si", name="Si")
                Sj = s1p.tile([P, LC], f32, tag="Sj", name="Sj")
                nc.gpsimd.tensor_mul(out=Si[:], in0=WT[0][:], in1=RZ[:, 0, :])
                nc.gpsimd.tensor_mul(out=Sj[:], in0=WT[1][:], in1=RZ[:, 1, :])
                nc.gpsimd.tensor_sub(out=Si[:], in0=Si[:], in1=Sj[:])
                S1 = s1p.tile([P, LC], f32, tag="S1", name="S1")
                nc.gpsimd.tensor_mul(out=S1[:], in0=SG[:], in1=Si[:])
                WS = s1p.tile([P, LC], bf16, tag="WS", name="WS")
                nc.gpsimd.tensor_add(out=WS[:], in0=WI[:], in1=WJ[:])
                S2C = s1p.tile([P, LC], f32, tag="S2C", name="S2C")
                nc.gpsimd.tensor_mul(out=S2C[:], in0=DIST[:], in1=WS[:])

                S2 = s1p.tile([P, LC], f32, tag="S2", name="S2")
                nc.scalar.activation(S2[:], S1[:], AF.Exp, scale=-1.0)
                nc.scalar.activation(S1[:], S2[:], AF.Ln, bias=1.0)
                S3 = s1p.tile([P, LC], f32, tag="S3", name="S3")
                nc.vector.tensor_scalar(
                    out=S3[:], in0=DIST[:], scalar1=1.0, scalar2=None, op0=ALU.min)

                S5 = s1p.tile([P, LC], f32, tag="S5", name="S5")
                nc.vector.scalar_tensor_tensor(
                    out=S5[:], in0=S1[:], scalar=1.0, in1=S2C[:],
                    op0=ALU.mult, op1=ALU.mult, accum_out=accL[c][:])
                nc.vector.scalar_tensor_tensor(
                    out=S1[:], in0=S3[:], scalar=1.0, in1=WS[:],
                    op0=ALU.mult, op1=ALU.mult, accum_out=accW[c][:])
                if c > 0:
                    nc.vector.tensor_add(out=accL[c][:], in0=accL[c][:],
                                         in1=accL[c - 1][:])
                    nc.vector.tensor_add(out=accW[c][:], in0=accW[c][:],
                                         in1=accW[c - 1][:])

            # staggered emission: engines run their streams in order, so
            # chunk c+1's front-half is emitted before chunk c's tail to
            # let the phases pipeline across engines.
            for c in range(NCHUNK):
                stage1(c)
                stage2(c)

            nc.sync.dma_start(out=OUT[:, 0:1], in_=accL[NCHUNK - 1][:])
            nc.sync.dma_start(out=OUT[:, 1:2], in_=accW[NCHUNK - 1][:])

    nc.compile()
    return nc


_NC_CACHE = {}


def _get_nc():
    if "nc" not in _NC_CACHE:
        _NC_CACHE["nc"] = _build()
    return _NC_CACHE["nc"]


def _prepare(inputs, targets, cluster_ids, sample_weight, pair_i, pair_j):
    import ml_dtypes

    x = np.ascontiguousarray(np.asarray(inputs), dtype=np.float32)
    t = np.asarray(targets)
    w = np.asarray(sample_weight, dtype=np.float32)
    pi = np.asarray(pair_i).astype(np.int64, copy=False)
    pj = np.asarray(pair_j).astype(np.int64, copy=False)

    li = x[pi]  # (NPAIRS, 5)
    lj = x[pj]
    lis = li[:, 1:5] - li[:, 0:1]  # l0-shift: softmax is shift-invariant
    ljs = lj[:, 1:5] - lj[:, 0:1]
    dy = (t[pi] - t[pj]).astype(np.float32)
    wi = w[pi]
    wj = w[pj]

    PL = P * L
    bf = ml_dtypes.bfloat16
    maps = []
    for k in range(NCORES):
        sl = slice(k * PC, (k + 1) * PC)
        A = np.zeros((P, NPLANE, L), dtype=bf)

        def put(plane, v):
            vv = np.zeros(PL, dtype=np.float32)
            vv[:PC] = v
            A[:, plane, :] = vv.reshape(P, L).astype(bf)

        for ccls in range(4):
            put(ccls, lis[sl][:, ccls])
            put(4 + ccls, ljs[sl][:, ccls])
        put(8, dy[sl])
        put(9, wi[sl])
        put(10, wj[sl])
        maps.append({"x": A})
    return maps


def _run(in_maps, trace=False, **kw):
    nc = _get_nc()
    return run_bass_kernel_spmd(nc, in_maps, list(range(NCORES)), trace=trace, **kw)


def kernel(inputs, targets, cluster_ids, sample_weight, pair_i, pair_j):
    in_maps = _prepare(inputs, targets, cluster_ids, sample_weight, pair_i, pair_j)
    res = _run(in_maps)
    tl = 0.0
    tw = 0.0
    for k in range(NCORES):
        o = res.results[k]["out"]
        tl += float(o[:, 0].sum(dtype=np.float64))
        tw += float(o[:, 1].sum(dtype=np.float64))
    # the 0.5 pair-weight factor cancels in the ratio; fold it into eps
    return np.float32(tl / (tw + 2 * EPS))
